# revision 6
# baseline (speedup 1.0000x reference)
"""Multi-head causal self-attention (RMSNorm + RoPE) Bass kernel for TRN2.

Problem: b=2, s=4096, dim=512, 8 heads, dh=64, fp32.
    out = softmax((rope(q) @ rope(k).T) / 8 + causal) @ v @ Wo
    with q/k/v = RMSNorm(x) @ W{q,k,v}.

Sharding (8 NeuronCores): core c handles batch c//4 and heads
{2*(c%4), 2*(c%4)+1} (tensor-parallel over heads, data-parallel over
batch).  Each core computes a row-parallel partial of the final Wo
projection; partials are summed on the host during unshard.

Per-core pipeline (all matmuls in float32r: 4x fp32 throughput,
~1.5e-4 max rel err):
  - RMSNorm in [token, dim] layout (ACT square+accum, sqrt, DVE recip)
  - PE-transpose x_norm to [dim, token]
  - q^T/k^T = W^T @ xn^T, plus a second projection with
    column-permuted weights (the rotate-half partner rows), so RoPE is
    3 aligned full-width DVE ops: q_rot = q*Tc + q_perm*Ts
  - v in [token, col] layout with an appended ones-column (gives the
    softmax denominator for free in the PV matmul)
  - attention in S^T(kt,qt) orientation: causal mask pre-loaded into
    PSUM via an identity matmul, exp on ScalarE (no max subtraction:
    scores are O(5), fp32-safe), PV accumulates out^T[dh+1, qt]
  - normalize by broadcasting 1/denom via a K=1 ones matmul
  - final^T[dim, qt] partial = Wo_head^T @ out_norm summed over the
    core's 2 heads in PSUM
"""

from contextlib import ExitStack

import numpy as np

import concourse.bacc as bacc
import concourse.mybir as mybir
import concourse.tile as tile

f32 = mybir.dt.float32
f32r = mybir.dt.float32r
AF = mybir.ActivationFunctionType

B, S, DIM, H, DH = 2, 4096, 512, 8, 64
HALF = DH // 2
NCORE = 8
HPC = 2                  # heads per core
COLS = HPC * DH          # 128
CJ = 512                 # qt chunk size
NJ = S // CJ             # 8
TT = S // 128            # 32 token tiles
TG = 4                   # token tiles per group (= one J chunk)
KC = DIM // 128          # 4 contraction chunks
EPS = 1e-6
ROPE_BASE = 1000000.0
NEG = -1e9

TRACE = False            # test harness can flip this
TRACE_KW = {}
LAST_RESULT = None       # BassKernelResults of the last run


class _K:
    """Holds nc + pools/tiles shared across phase helpers."""


def _norm_and_transpose(k, J):
    """RMSNorm 4 token tiles of chunk J, PE-transpose into xnT."""
    nc = k.nc
    Js = slice(J * CJ, (J + 1) * CJ)
    xns = []
    for i in range(TG):
        tt = J * TG + i
        x_t = k.xp.tile([128, DIM], f32, tag="x")
        nc.sync.dma_start(out=x_t, in_=k.xb[tt * 128:(tt + 1) * 128, :])
        sq = k.sqp.tile([128, DIM], f32, tag="sq")
        ssum = k.stp.tile([128, 1], f32, tag="ssum")
        nc.scalar.activation(sq, x_t, AF.Square, accum_out=ssum)
        srt = k.stp.tile([128, 1], f32, tag="srt")
        nc.scalar.activation(srt, ssum, AF.Sqrt, scale=1.0 / DIM,
                             bias=k.eps_sb[:, 0:1])
        rstd = k.stp.tile([128, 1], f32, tag="rstd")
        nc.vector.reciprocal(rstd, srt)
        xn = k.xnp.tile([128, DIM], f32r, tag="xn")
        with nc.allow_low_precision(reason="f32r is fp32-width"):
            nc.vector.tensor_scalar_mul(xn, x_t, rstd)
        xns.append(xn)
    for dc in range(KC):
        tr = k.psA.tile([128, CJ], f32r, tag="psa")
        for i in range(TG):
            nc.tensor.transpose(tr[:, i * 128:(i + 1) * 128],
                                xns[i][:, dc * 128:(dc + 1) * 128], k.ident)
        with nc.allow_low_precision(reason="f32r is fp32-width"):
            nc.vector.tensor_copy(k.xnT[dc][:, Js], tr)


def _qkv(k, J):
    """q^T/k^T projections (+permuted twin) with fused RoPE; v tiles."""
    nc = k.nc
    Js = slice(J * CJ, (J + 1) * CJ)
    tc_sb = k.tabs.tile([128, CJ], f32, tag="tc")
    nc.sync.dma_start(out=tc_sb, in_=k.tcb[:, Js])
    ts_sb = k.tabs.tile([128, CJ], f32, tag="ts")
    nc.sync.dma_start(out=ts_sb, in_=k.tsb[:, Js])
    for dst, w_sb, wp_sb in ((k.qTr, k.wq_sb, k.wqp_sb),
                             (k.kTr, k.wk_sb, k.wkp_sb)):
        a_ps = k.psB.tile([128, CJ], f32, tag="psb")
        ap_ps = k.psB.tile([128, CJ], f32, tag="psb")
        for kc in range(KC):
            nc.tensor.matmul(a_ps, w_sb[:, kc, :], k.xnT[kc][:, Js],
                             start=(kc == 0), stop=(kc == KC - 1))
        for kc in range(KC):
            nc.tensor.matmul(ap_ps, wp_sb[:, kc, :], k.xnT[kc][:, Js],
                             start=(kc == 0), stop=(kc == KC - 1))
        t1 = k.rtmp.tile([128, CJ], f32, tag="t1")
        nc.vector.tensor_mul(t1, a_ps, tc_sb)
        t2 = k.rtmp.tile([128, CJ], f32, tag="t2")
        nc.vector.tensor_mul(t2, ap_ps, ts_sb)
        with nc.allow_low_precision(reason="f32r is fp32-width"):
            nc.vector.tensor_add(dst[:, Js], t1, t2)
    for i in range(TG):
        tt = J * TG + i
        v_ps = k.psB.tile([128, COLS], f32, tag="psb")
        for kc in range(KC):
            nc.tensor.matmul(v_ps, k.xnT[kc][:, tt * 128:(tt + 1) * 128],
                             k.wv_sb[:, kc, :],
                             start=(kc == 0), stop=(kc == KC - 1))
        va = k.v_all[tt]
        with nc.allow_low_precision(reason="f32r is fp32-width"):
            nc.vector.tensor_copy(va[:, 0:DH], v_ps[:, 0:DH])
            nc.vector.tensor_copy(va[:, DH + 1:2 * DH + 1],
                                  v_ps[:, DH:2 * DH])


def _attn_head(k, J, h):
    """Causal attention for qt chunk J, head h -> normalized out^T."""
    nc = k.nc
    Js = slice(J * CJ, (J + 1) * CJ)
    hs = slice(h * DH, (h + 1) * DH)
    outT = k.psO.tile([DH + 1, CJ], f32, tag="pso")
    npairs = 2 * (J + 1)
    first_mm = True
    for p in range(npairs):
        S_ps = k.psA.tile([128, 2 * CJ], f32, tag="psa")
        for half in range(2):
            I = 2 * p + half
            dd = I - 4 * J
            sl = slice(half * CJ, (half + 1) * CJ)
            if 0 <= dd <= 3:
                nc.tensor.matmul(S_ps[:, sl], k.ident,
                                 k.mk_sb[:, 384 - 128 * dd:896 - 128 * dd],
                                 start=True, stop=False)
                nc.tensor.matmul(S_ps[:, sl], k.kTr[hs, I * 128:(I + 1) * 128],
                                 k.qTr[hs, Js], start=False, stop=True)
            else:
                nc.tensor.matmul(S_ps[:, sl], k.kTr[hs, I * 128:(I + 1) * 128],
                                 k.qTr[hs, Js], start=True, stop=True)
        E = k.ep.tile([128, 2 * CJ], f32r, tag="E")
        nc.scalar.activation(E, S_ps, AF.Exp)
        for half in range(2):
            I = 2 * p + half
            last_mm = (p == npairs - 1 and half == 1)
            nc.tensor.matmul(outT,
                             k.v_all[I][:, h * (DH + 1):(h + 1) * (DH + 1)],
                             E[:, half * CJ:(half + 1) * CJ],
                             start=first_mm, stop=last_mm,
                             skip_group_check=True)
            first_mm = False
    recip = k.rp.tile([DH + 1, CJ], f32r, tag="recip")
    with nc.allow_low_precision(reason="f32r is fp32-width"):
        nc.vector.reciprocal(recip[DH:DH + 1, :], outT[DH:DH + 1, :])
    D_ps = k.psB.tile([DH, CJ], f32, tag="psb")
    nc.tensor.matmul(D_ps, k.ones_t[DH:DH + 1, :], recip[DH:DH + 1, :],
                     start=True, stop=True)
    D_sb = k.dp.tile([DH, CJ], f32, tag="dsb")
    nc.vector.tensor_copy(D_sb, D_ps)
    on_t = k.onp.tile([DH, CJ], f32r, tag="on")
    with nc.allow_low_precision(reason="f32r is fp32-width"):
        nc.vector.tensor_mul(on_t, outT[0:DH, :], D_sb)
    return on_t


def _final(k, J, outn):
    """Partial Wo projection for chunk J -> DRAM."""
    nc = k.nc
    Js = slice(J * CJ, (J + 1) * CJ)
    for odc in range(KC):
        fin_ps = k.psB.tile([128, CJ], f32, tag="psb")
        for h in range(HPC):
            nc.tensor.matmul(fin_ps, k.wo_sb[:, h, odc * 128:(odc + 1) * 128],
                             outn[h], start=(h == 0), stop=(h == HPC - 1))
        fin_sb = k.fpp.tile([128, CJ], f32, tag="fin")
        nc.vector.tensor_copy(fin_sb, fin_ps)
        nc.sync.dma_start(out=k.outp[odc * 128:(odc + 1) * 128, Js],
                          in_=fin_sb)


def _build():
    nc = bacc.Bacc("TRN2", target_bir_lowering=False, debug=False)
    k = _K()
    k.nc = nc

    k.xb = nc.declare_dram_parameter("xb", [S, DIM], f32, isOutput=False)
    wq = nc.declare_dram_parameter("wq", [DIM, COLS], f32r, isOutput=False)
    wqp = nc.declare_dram_parameter("wqp", [DIM, COLS], f32r, isOutput=False)
    wk = nc.declare_dram_parameter("wk", [DIM, COLS], f32r, isOutput=False)
    wkp = nc.declare_dram_parameter("wkp", [DIM, COLS], f32r, isOutput=False)
    wv = nc.declare_dram_parameter("wv", [DIM, COLS], f32r, isOutput=False)
    wo = nc.declare_dram_parameter("wo", [HPC, DH, DIM], f32r, isOutput=False)
    k.tcb = nc.declare_dram_parameter("tcb", [128, S], f32, isOutput=False)
    k.tsb = nc.declare_dram_parameter("tsb", [128, S], f32, isOutput=False)
    mk = nc.declare_dram_parameter("mk", [128, 896], f32r, isOutput=False)
    ident_in = nc.declare_dram_parameter("ident_in", [128, 128], f32r,
                                         isOutput=False)
    ones_in = nc.declare_dram_parameter("ones_in", [DH + 1, DH], f32r,
                                        isOutput=False)
    vones = nc.declare_dram_parameter("vones", [128, 2], f32r, isOutput=False)
    k.outp = nc.declare_dram_parameter("outp", [DIM, S], f32, isOutput=True)

    with ExitStack() as ctx:
        tc = ctx.enter_context(tile.TileContext(nc))
        pool = lambda name, bufs, **kw: ctx.enter_context(
            tc.tile_pool(name=name, bufs=bufs, **kw))
        consts = pool("consts", 1)
        k.xp = pool("xp", 4)
        k.sqp = pool("sq", 1)
        k.stp = pool("st", 8)
        k.xnp = pool("xn", 6)
        xnTp = pool("xnT", 1)
        k.tabs = pool("tabs", 3)
        k.rtmp = pool("rtmp", 3)
        qkp = pool("qk", 1)
        vp = pool("vp", 1)
        k.ep = pool("ep", 2)
        k.rp = pool("rp", 2)
        k.dp = pool("dp", 2)
        k.onp = pool("on", 2)
        k.fpp = pool("fp", 2)
        k.psA = pool("psA", 2, space="PSUM")
        k.psB = pool("psB", 3, space="PSUM")
        k.psO = pool("psO", 1, space="PSUM")

        k.wq_sb = consts.tile([128, KC, COLS], f32r, tag="wq")
        k.wqp_sb = consts.tile([128, KC, COLS], f32r, tag="wqp")
        k.wk_sb = consts.tile([128, KC, COLS], f32r, tag="wk")
        k.wkp_sb = consts.tile([128, KC, COLS], f32r, tag="wkp")
        k.wv_sb = consts.tile([128, KC, COLS], f32r, tag="wv")
        for t, d in ((k.wq_sb, wq), (k.wqp_sb, wqp), (k.wk_sb, wk),
                     (k.wkp_sb, wkp), (k.wv_sb, wv)):
            nc.sync.dma_start(out=t, in_=d.rearrange("(kc k) m -> k kc m",
                                                     k=128))
        k.wo_sb = consts.tile([DH, HPC, DIM], f32r, tag="wo")
        nc.sync.dma_start(out=k.wo_sb, in_=wo.rearrange("h k n -> k h n"))
        k.mk_sb = consts.tile([128, 896], f32r, tag="mk")
        nc.sync.dma_start(out=k.mk_sb, in_=mk[:, :])
        k.ident = consts.tile([128, 128], f32r, tag="ident")
        nc.sync.dma_start(out=k.ident, in_=ident_in[:, :])
        k.ones_t = consts.tile([DH + 1, DH], f32r, tag="ones")
        nc.sync.dma_start(out=k.ones_t, in_=ones_in[:, :])
        k.eps_sb = consts.tile([128, 1], f32, tag="eps")
        nc.vector.memset(k.eps_sb, EPS)

        k.xnT = [xnTp.tile([128, S], f32r, tag=f"xnT{dc}", name=f"xnT{dc}")
                 for dc in range(KC)]
        k.qTr = qkp.tile([128, S], f32r, tag="qTr")
        k.kTr = qkp.tile([128, S], f32r, tag="kTr")
        k.v_all = [vp.tile([128, HPC * (DH + 1)], f32r, tag=f"v{i}", name=f"v{i}")
                   for i in range(TT)]
        for i in range(TT):
            nc.sync.dma_start(out=k.v_all[i][:, DH:DH + 1], in_=vones[:, 0:1])
            nc.sync.dma_start(out=k.v_all[i][:, 2 * DH + 1:2 * DH + 2],
                              in_=vones[:, 1:2])

        for J in range(NJ):
            _norm_and_transpose(k, J)
            _qkv(k, J)
            outn = {h: _attn_head(k, J, h) for h in range(HPC)}
            _final(k, J, outn)

    nc.compile()
    return nc


def _rope_tables():
    """cos/sin tables bit-matched to the jax fp32 reference (on CPU)."""
    import jax
    import jax.numpy as jnp
    cpu = jax.devices("cpu")[0]
    with jax.default_device(cpu):
        theta = 1.0 / ROPE_BASE ** (2 * jnp.arange(HALF) / DH)
        angles = jnp.arange(S)[:, None] * theta[None, :]
        cos = np.asarray(jnp.cos(angles), dtype=np.float32)
        sin = np.asarray(jnp.sin(angles), dtype=np.float32)
    return cos, sin


_NC = None


def kernel(x, scale, Wq, Wk, Wv, Wo):
    global _NC, LAST_RESULT
    from concourse.bass_utils import run_bass_kernel_spmd

    x = np.ascontiguousarray(np.asarray(x, dtype=np.float32))
    scale = np.asarray(scale, dtype=np.float32)
    Wq = np.asarray(Wq, dtype=np.float32)
    Wk = np.asarray(Wk, dtype=np.float32)
    Wv = np.asarray(Wv, dtype=np.float32)
    Wo = np.asarray(Wo, dtype=np.float32)

    if _NC is None:
        _NC = _build()

    cos, sin = _rope_tables()          # [S, 32] each
    r32 = np.arange(128) % HALF
    Tc = np.ascontiguousarray(cos.T[r32, :])              # [128, S]
    sgn = np.where((np.arange(128) % DH) < HALF, -1.0, 1.0).astype(np.float32)
    Ts = np.ascontiguousarray(sin.T[r32, :] * sgn[:, None])

    mk = np.full((128, 896), NEG, dtype=np.float32)
    pp = np.arange(128)[:, None]
    ff = np.arange(512)[None, :]
    mk[:, 384:] = np.where(pp <= ff, np.float32(0.0), np.float32(NEG))

    mloc = np.arange(COLS)
    sig = (mloc // DH) * DH + ((mloc % DH) + HALF) % DH

    wq_eff = (Wq * scale[:, None]) * np.float32(0.125)
    wk_eff = Wk * scale[:, None]
    wv_eff = Wv * scale[:, None]

    in_maps = []
    for c in range(NCORE):
        beta, hp = divmod(c, 4)
        cs = slice(hp * COLS, (hp + 1) * COLS)
        wq_c = np.ascontiguousarray(wq_eff[:, cs])
        wk_c = np.ascontiguousarray(wk_eff[:, cs])
        in_maps.append({
            "xb": x[beta],
            "wq": wq_c,
            "wqp": np.ascontiguousarray(wq_c[:, sig]),
            "wk": wk_c,
            "wkp": np.ascontiguousarray(wk_c[:, sig]),
            "wv": np.ascontiguousarray(wv_eff[:, cs]),
            "wo": np.ascontiguousarray(Wo[cs, :].reshape(HPC, DH, DIM)),
            "tcb": Tc,
            "tsb": Ts,
            "mk": mk,
            "ident_in": np.eye(128, dtype=np.float32),
            "ones_in": np.ones((DH + 1, DH), dtype=np.float32),
            "vones": np.ones((128, 2), dtype=np.float32),
        })

    res = run_bass_kernel_spmd(_NC, in_maps, list(range(NCORE)),
                               trace=TRACE, **TRACE_KW)
    LAST_RESULT = res

    out = np.zeros((B, S, DIM), dtype=np.float32)
    for c in range(NCORE):
        out[c // 4] += res.results[c]["outp"].T
    return out
